# revision 1
# baseline (speedup 1.0000x reference)
"""Bass/Trainium2 kernel for nn_ContrastiveAlignmentLoss.

reference math (B=256, N=512):
    global_sim = graph.mean(axis=(1, 2))                    # [B]
    sim        = outer(global_sim, global_sim)              # [B, B]
    same       = labels[:, None] == labels[None, :]
    pair_loss  = where(same, relu(0.5 - sim), relu(sim - 0.5))
    loss       = sum(triu(pair_loss, k=1)) / (B*(B-1)/2)

Distribution: data-parallel over B across 8 NeuronCores. Each core
sum-pools its 32 relation graphs (the memory-bound part: 32 MiB/core),
AllGathers the tiny [B] raw sums, and computes the pairwise loss
replicated (the 1/N^2 mean scaling is folded into the pairwise phase:
sim = (s_i*s_j)/N^4). pair_loss is symmetric, so sum over i<j equals
(sum over all i,j - sum over diagonal) / 2, with
    pair(i,j) = relu(d) - same*d,   d = sim - 0.5
    pair(i,i) = relu(0.5 - g_i^2) = -min(g_i^2 - 0.5, 0)

Perf notes:
- load DMAs alternate between BOTH HWDGE rings (qSPDynamicHW on sync,
  qActDynamicHW on scalar); all free-axis reductions run on DVE alone
  (34us busy inside a ~100us window) so the ACT engine only issues DMA
  triggers and neither ring serializes behind compute.
- a 32-byte warm-up AllGather fired mid-load re-synchronizes the ranks
  so the real AllGather's ncfw wakeup + entry barrier cost ~6-9us
  instead of 20-35us.
- the cross-partition matmul + cc_in staging is split 16/12/4 so only
  the last 4 columns (reduce + [1,4] matmul + copy + 16B DMA) sit on
  the critical path between the last load packet and the AllGather
  trigger.
- pairwise phase: d = INV2*g_i*g_j - 0.5 comes straight out of a K=2
  PE outer product (row1 of lhsT/rhs is a const -0.5/ones pair), so no
  partition broadcast and no strided column gather; DVE just does
  same*d and relu(d)-same*d with accum_out. Diagonal terms run on ACT
  (Square then Relu(-x) with accum_out) in parallel.
"""

import numpy as np

import concourse.bacc as bacc
import concourse.mybir as mybir
import concourse.tile as tile
from concourse.bass_utils import run_bass_kernel_spmd

N_CORES = 8
B = 256
N = 512
BS = B // N_CORES          # 32 graphs per core
NN = N * N                 # 262144 elements per graph
P = 128                    # SBUF partitions
FREE = NN // P             # 2048 f32 per partition per graph
MARGIN = 0.5
NUM_PAIRS = B * (B - 1) // 2
INV2 = 1.0 / (float(NN) * float(NN))   # folds the two mean divisions
KK = 0.5 / NUM_PAIRS

# knobs for test.py (harness never touches these)
TRACE = False
TRACE_DIR = None
TRACE_CORES = None
LAST_EXEC_NS = None
LAST_RESULTS = None

_CACHED_NC = None


def build_body(tc, loss_ap, graph_ap, labels_ap):
    """Emit the per-core program. graph_ap: [BS, N, N] f32 shard,
    labels_ap: [1, B] f32 full labels, loss_ap: [1, 1] f32 out."""
    nc = tc.nc
    f32 = mybir.dt.float32
    X = mybir.AxisListType.X
    ALU = mybir.AluOpType
    Copy = mybir.ActivationFunctionType.Copy

    # [BS, N, N] -> [BS, P, FREE]; per partition a contiguous 8 KiB run
    gview = graph_ap.rearrange("b n m -> b (n m)").rearrange(
        "b (p c) -> b p c", p=P
    )

    with (
        tc.tile_pool(name="io", bufs=12) as io_pool,
        tc.tile_pool(name="acc", bufs=1) as acc,
        tc.tile_pool(name="ps1", bufs=1, space="PSUM") as ps1,
        tc.tile_pool(name="psd", bufs=2, space="PSUM") as psd,
        tc.tile_pool(name="dram", bufs=1, space="DRAM") as dram,
    ):
        S = acc.tile([P, BS + 1], f32, tag="S")       # per-graph column sums
        ones_col = acc.tile([P, 1], f32, tag="ones_col")
        nc.vector.memset(ones_col[:], 1.0)

        # pairwise-phase constant rows (row1 of the K=2 outer product)
        # memset both partitions with the const row value; row0 is later
        # overwritten by the gathered-g writes (single-partition writes must
        # start at partition 0, so row1 can't be set directly).
        rhs2 = acc.tile([2, B], f32, tag="rhs2")      # row0: INV2*g, row1: 1
        nc.vector.memset(rhs2[:], 1.0)
        combo = acc.tile([2, B], f32, tag="combo")    # row0: raw g, row1: -0.5
        nc.vector.memset(combo[:], -MARGIN)           # lhsT chunks slice this

        # labels prep — independent of the graph data, runs during load
        lab_row = acc.tile([1, B], f32, tag="lab_row")
        nc.sync.dma_start(lab_row[:], labels_ap)
        lb = acc.tile([P, B], f32, tag="lb")
        nc.gpsimd.partition_broadcast(lb[:], lab_row[:])
        sames = []
        for c in range(2):
            lab_col = acc.tile([P, 1], f32, tag=f"lab_col{c}")
            nc.scalar.dma_start(lab_col[:], labels_ap[0, c * P : (c + 1) * P])
            same = acc.tile([P, B], f32, tag=f"same{c}")
            nc.vector.tensor_scalar(
                same[:], lb[:], lab_col[:], None, ALU.is_equal
            )
            sames.append(same)

        # collective buffers (Shared outputs: HBM-HBM AllGather fast path)
        warm_in0 = dram.tile([1, 8], f32, tag="warm_in0")
        warm_out0 = dram.tile([N_CORES, 8], f32, tag="warm_out0", addr_space="Shared")
        cc_in = dram.tile([1, BS], f32, tag="cc_in")
        cc_out = dram.tile([N_CORES, BS], f32, tag="cc_out", addr_space="Shared")

        # warm-up collective #1, fired before the load even starts: absorbs
        # the CC entry barrier + ncfw wakeup so the later collectives are
        # not serialized behind them. Content is irrelevant (labels bytes).
        nc.sync.dma_start(warm_in0[:], labels_ap[:, 0:8])
        nc.gpsimd.collective_compute(
            "AllGather",
            ALU.bypass,
            replica_groups=[list(range(N_CORES))],
            ins=[warm_in0[:]],
            outs=[warm_out0[:]],
        )

        ps_g = ps1.tile([1, BS + 1], f32, tag="ps_g")
        SQ = float(np.sqrt(INV2))
        g_sb = acc.tile([1, BS], f32, tag="g_sb")

        def stage(lo, hi):
            # cross-partition sum for graphs [lo,hi) + stage into cc_in
            nc.tensor.matmul(ps_g[:, lo:hi], ones_col[:], S[:, lo:hi])
            nc.vector.tensor_scalar(
                g_sb[:, lo:hi], ps_g[:, lo:hi], SQ, None, ALU.mult
            )
            nc.sync.dma_start(cc_in[:, lo:hi], g_sb[:, lo:hi])

        # ---- heavy phase: sum-pool each graph (DMA-bound) ----
        # loads alternate across both HWDGE rings; every reduce on DVE.
        rings = [nc.sync, nc.scalar]
        H = FREE // 2
        for b in range(BS):
            t = io_pool.tile([P, FREE], f32, tag="gtile")
            if b < BS - 1:
                rings[b % 2].dma_start(t[:], gview[b])
                # cross-assigned reduces: sync-ring tiles reduce on ACT,
                # scalar-ring tiles on DVE, so neither engine ever waits on
                # a tile from the ring it feeds and each stays ~50% busy.
                if b % 2 == 0:
                    nc.scalar.activation(
                        t[:], t[:], Copy, accum_out=S[:, b : b + 1]
                    )
                else:
                    nc.vector.reduce_sum(S[:, b : b + 1], t[:], axis=X)
            else:
                # last graph: halves on both rings; DVE and ACT reduce the
                # halves in parallel so only ~1.1us sits after the last
                # packet instead of a 2.3us full-tile reduce.
                nc.sync.dma_start(t[:, 0:H], gview[b][:, 0:H])
                nc.scalar.dma_start(t[:, H:FREE], gview[b][:, H:FREE])
                nc.vector.reduce_sum(S[:, b : b + 1], t[:, 0:H], axis=X)
                nc.scalar.activation(
                    t[:, H:FREE], t[:, H:FREE], Copy,
                    accum_out=S[:, BS : BS + 1],
                )
            if b == 15:
                stage(0, 16)
            elif b == 27:
                stage(16, 28)
            elif b == 30:
                stage(28, 31)
        # merge the two half-sums of the last graph: matmul over both
        # columns, then scale-and-accumulate into one cc_in element.
        nc.tensor.matmul(ps_g[:, 31:33], ones_col[:], S[:, 31:33])
        gtmp = acc.tile([1, 2], f32, tag="gtmp")
        nc.vector.tensor_scalar(
            gtmp[:], ps_g[:, 31:33], SQ, None, ALU.mult, op1=ALU.add,
            accum_out=g_sb[:, 31:32],
        )
        nc.sync.dma_start(cc_in[:, 31:32], g_sb[:, 31:32])

        # ---- all-gather the [BS] raw sums -> [B] ----
        nc.gpsimd.collective_compute(
            "AllGather",
            ALU.bypass,
            replica_groups=[list(range(N_CORES))],
            ins=[cc_in[:]],
            outs=[cc_out[:]],
        )

        # gathered pre-scaled sums ghat = sqrt(INV2)*sum, flat [B] in DRAM;
        # two parallel DMAs fill both matmul operand rows, no scale op.
        flat = cc_out[:].rearrange("r b -> (r b)")
        graw = combo[0:1, :]
        nc.sync.dma_start(graw, flat[None, :])
        nc.scalar.dma_start(rhs2[0:1, :], flat[None, :])

        # diagonal terms on ACT: dneg = sum(relu(0.5*KK - gg2)),
        # gg2 = (g*sqrt(INV2*KK))^2; loss uses -dneg.
        sq = float(np.sqrt(KK))
        gg2 = acc.tile([1, B], f32, tag="gg2")
        nc.scalar.activation(
            gg2[:], graw, mybir.ActivationFunctionType.Square, scale=sq
        )
        dneg = acc.tile([1, 1], f32, tag="dneg")
        relu_tmp = acc.tile([1, B], f32, tag="relu_tmp")
        diag_bias = acc.tile([1, 1], f32, tag="diag_bias")
        nc.vector.memset(diag_bias[:], MARGIN * KK)
        nc.scalar.activation(
            relu_tmp[:], gg2[:], mybir.ActivationFunctionType.Relu,
            scale=-1.0, bias=diag_bias[:], accum_out=dneg[:],
        )

        # ---- pairwise loss: d straight out of a K=2 PE outer product ----
        CS = acc.tile([P, 2], f32, tag="CS")
        for c in range(2):
            dps = psd.tile([P, B], f32, tag="dps")   # d = INV2*gi*gj - 0.5
            nc.tensor.matmul(dps[:], combo[:, c * P : (c + 1) * P], rhs2[:])
            sd = acc.tile([P, B], f32, tag=f"sd{c}")    # same * d
            nc.vector.tensor_tensor(sd[:], sames[c][:], dps[:], ALU.mult)
            pair = acc.tile([P, B], f32, tag=f"pair{c}")  # relu(d) - sd
            nc.vector.scalar_tensor_tensor(
                pair[:], dps[:], 0.0, sd[:], ALU.max, ALU.subtract,
                accum_out=CS[:, c : c + 1],
            )

        # total = sum all (i,j); loss = KK*total - dneg
        ps_tot = ps1.tile([1, 2], f32, tag="ps_tot")
        nc.tensor.matmul(ps_tot[:], ones_col[:], CS[:])
        tk = acc.tile([1, 2], f32, tag="tk")
        totk = acc.tile([1, 1], f32, tag="totk")
        nc.vector.tensor_scalar(
            tk[:], ps_tot[:], KK, None, ALU.mult, op1=ALU.add,
            accum_out=totk[:],
        )
        res = acc.tile([1, 1], f32, tag="res")
        nc.vector.tensor_tensor(res[:], totk[:], dneg[:], ALU.subtract)
        nc.sync.dma_start(loss_ap, res[:])


def _build():
    global _CACHED_NC
    if _CACHED_NC is not None:
        return _CACHED_NC
    nc = bacc.Bacc(
        "TRN2", target_bir_lowering=False, debug=False, num_devices=N_CORES
    )
    g_in = nc.dram_tensor(
        "graph", [BS, N, N], mybir.dt.float32, kind="ExternalInput"
    )
    lab_in = nc.dram_tensor(
        "labels_f32", [1, B], mybir.dt.float32, kind="ExternalInput"
    )
    out = nc.dram_tensor("loss", [1, 1], mybir.dt.float32, kind="ExternalOutput")
    with tile.TileContext(nc) as tc:
        build_body(tc, out.ap(), g_in.ap(), lab_in.ap())
    nc.compile()
    _CACHED_NC = nc
    return nc


def kernel(graph, labels):
    global LAST_EXEC_NS, LAST_RESULTS
    graph = np.ascontiguousarray(np.asarray(graph), dtype=np.float32)
    labels_f32 = np.asarray(labels).astype(np.float32).reshape(1, B)
    assert graph.shape == (B, N, N)

    nc = _build()
    in_maps = [
        {"graph": graph[c * BS : (c + 1) * BS], "labels_f32": labels_f32}
        for c in range(N_CORES)
    ]
    res = run_bass_kernel_spmd(
        nc,
        in_maps,
        core_ids=list(range(N_CORES)),
        trace=TRACE,
        tmpdir=TRACE_DIR,
        trace_cores=TRACE_CORES,
    )
    LAST_RESULTS = res
    LAST_EXEC_NS = res.exec_time_ns
    return np.asarray(res.results[0]["loss"][0, 0], dtype=np.float32)

